# revision 1
# baseline (speedup 1.0000x reference)
"""Trainium2 Bass kernel for nn_ChannelSparseConnectionEinsum (moe_routing).

Data-parallel over tokens: 8 cores x 512 tokens. Key reformulation: the
top-k gather/scatter of the reference is an elementwise mask (scatter-add
lands back at the gathered indices, which are unique), so

  out = full * (Em_o / D_o)  +  (x * (Em_i / D_i)) @ W  +  bias

where Em = exp(logits) with everything except the top-32 entries per row
zeroed (computed exactly with 4 rounds of DVE max8 + match_replace), and
D = row-sum of exp(logits) (softmax denominator, unnormalized-exp form).

BatchNorm is in training mode over ALL 4096 tokens -> partial sums are
all-reduced across the 8 cores (tiny collective). conv bias cancels in BN
(shift invariance) and is dropped. Gating conv is computed as a dense
matmul with the host-scattered [C1, G] weight; everything runs in f32.

Self-contained: hardcodes B=4, L=1024, C1=C2=1024, K=32, 8 cores.
"""

import numpy as np

import concourse.bacc as bacc
import concourse.bass as bass
import concourse.mybir as mybir
from concourse.bass_utils import run_bass_kernel_spmd
from concourse.masks import make_identity
from concourse.tile import TileContext

F32 = mybir.dt.float32
F32R = mybir.dt.float32r
USE_F32R = False  # top V2 lever: f32r value-path matmuls (needs rounded operand tiles)
ALU = mybir.AluOpType
AF = mybir.ActivationFunctionType

B, L, C1, C2 = 4, 1024, 1024, 1024
BN_EPS = 1e-5
G = C1 // 4
N_CORES = 8
TPC_PROD = (B * L) // N_CORES  # 512 tokens per core in production


def build_module(n_cores=N_CORES, tpc=TPC_PROD, use_collective=True, reps=1):
    """Build the per-core SPMD Bass module (same program on every core)."""
    nc = bacc.Bacc("TRN2", num_devices=n_cores, name="csce")
    NTT = tpc // 128        # token tiles
    KT = C1 // 128          # c1 contraction tiles (8)
    GT = G // 128           # g tiles (2)
    NTOT = float(n_cores * tpc)

    xT = nc.dram_tensor("xT", [C1, tpc], F32, kind="ExternalInput")
    w = nc.dram_tensor("w", [C1, C2], F32, kind="ExternalInput")
    at_o = nc.dram_tensor("at_o", [128, 256], F32, kind="ExternalInput")
    at_i = nc.dram_tensor("at_i", [128, 256], F32, kind="ExternalInput")
    ut_o = nc.dram_tensor("ut_o", [G, C2], F32, kind="ExternalInput")
    ut_i = nc.dram_tensor("ut_i", [G, C1], F32, kind="ExternalInput")
    gam_d = nc.dram_tensor("gam", [2 * GT, 128], F32, kind="ExternalInput")
    bet_d = nc.dram_tensor("bet", [2 * GT, 128], F32, kind="ExternalInput")
    lb_o = nc.dram_tensor("lb_o", [1, C2], F32, kind="ExternalInput")
    lb_i = nc.dram_tensor("lb_i", [1, C1], F32, kind="ExternalInput")
    bias_r = nc.dram_tensor("bias_r", [1, C2], F32, kind="ExternalInput")
    out_d = nc.dram_tensor("out", [tpc, C2], F32, kind="ExternalOutput")
    if use_collective:
        cc_in = nc.dram_tensor("cc_in", [128, 8], F32, kind="Internal")
        cc_out = nc.dram_tensor("cc_out", [128, 8], F32, kind="Internal",
                                addr_space="Shared")

    with TileContext(nc) as tc:
        with (
            tc.tile_pool(name="const", bufs=1) as cpool,
            tc.tile_pool(name="utp", bufs=2) as utpool,
            tc.tile_pool(name="small", bufs=2) as spool,
            tc.tile_pool(name="persist", bufs=1) as ppool,
            tc.tile_pool(name="ep", bufs=2) as ep,
            tc.tile_pool(name="fullp", bufs=4) as fullp,
            tc.tile_pool(name="wk", bufs=2) as wk,
            tc.tile_pool(name="ps", bufs=2, space="PSUM") as ps,
            tc.tile_pool(name="pstr", bufs=2, space="PSUM") as pstr,
        ):
            # ---------------- constants ----------------
            w_sb = []
            for k in range(KT):
                t = cpool.tile([128, C2], F32, tag=f"w{k}", name=f"w{k}")
                nc.sync.dma_start(t, w[128 * k:128 * (k + 1), :])
                w_sb.append(t)
            xT_all = cpool.tile([128, KT * tpc], F32, tag="xT_all")
            nc.sync.dma_start(
                xT_all.rearrange("p (k t) -> p k t", k=KT),
                xT.ap().rearrange("(k p) t -> p k t", p=128))
            xs_all = ppool.tile([128, KT * tpc], F32, tag="xs_all")

            def xtile(buf, k, lo, hi):
                return buf[:, k * tpc + lo:k * tpc + hi]

            at_all = {}
            for br, src in (("o", at_o), ("i", at_i)):
                t = cpool.tile([128, 256], F32, tag=f"at_all{br}",
                               name=f"at_all{br}")
                nc.sync.dma_start(t, src.ap())
                at_all[br] = t
            ut_i_sb = []
            for g in range(GT):
                t = utpool.tile([128, C2], F32, tag="ut", name=f"ut_i{g}")
                nc.sync.dma_start(t, ut_i[128 * g:128 * (g + 1), :])
                ut_i_sb.append(t)
            gam_all = spool.tile([128, 2 * GT], F32, tag="gam")
            bet_all = spool.tile([128, 2 * GT], F32, tag="bet")
            nc.sync.dma_start(gam_all, gam_d.ap().rearrange("a b -> b a"))
            nc.sync.dma_start(bet_all, bet_d.ap().rearrange("a b -> b a"))
            rows = {}
            for name, d in (("lb_o", lb_o), ("lb_i", lb_i), ("bias_r", bias_r)):
                t = spool.tile([1, C2], F32, tag=name, name=name)
                nc.sync.dma_start(t, d[:, :])
                rows[name] = t
            ident = cpool.tile([128, 128], F32, tag="ident")
            make_identity(nc, ident)
            ones_row = spool.tile([1, 128], F32, tag="ones_row")
            nc.vector.memset(ones_row, 1.0)

            for _rep in range(reps):
                if _rep:
                    tc.no_sync_barrier()
                # -------- stage A: conv (transposed layout) + BN partial sums ----
                # stats cols: [S1 o0,o1,i0,i1 | S2 o0,o1,i0,i1]
                stats = spool.tile([128, 8], F32, tag="stats")
                xcT = {}
                for bi, br in enumerate(("o", "i")):
                    xcT[br] = []
                    for g in range(GT):
                        col = 2 * bi + g
                        pc = ps.tile([128, C2], F32, tag="pacc", name="pc")[:, :tpc]
                        for jj in range(4):
                            k = 4 * g + jj
                            nc.tensor.matmul(
                                pc[32 * jj:32 * (jj + 1), :],
                                at_all[br][:, 32 * k:32 * (k + 1)],
                                xtile(xT_all, k, 0, tpc),
                                start=True, stop=True,
                                tile_position=(0, 32 * jj))
                        xc = ppool.tile([128, tpc], F32, tag=f"xcT{br}{g}",
                                        name=f"xcT{br}{g}")
                        nc.scalar.activation(xc, pc, AF.Copy,
                                             accum_out=stats[:, col:col + 1])
                        sq = wk.tile([128, tpc], F32, tag="sq", name="sq", bufs=1)
                        nc.scalar.activation(sq, xc, AF.Square,
                                             accum_out=stats[:, 4 + col:5 + col])
                        xcT[br].append(xc)

                # -------- stage F: full = x @ W (fills PE during rounds_i) -----
                full_sb = []
                for t in range(NTT):
                    pf = ps.tile([128, C2], F32, tag="pacc", name="pf")
                    for ch in range(2):
                        cs = slice(512 * ch, 512 * (ch + 1))
                        for k in range(KT):
                            lh = xtile(xT_all, k, 128 * t, 128 * (t + 1))
                            rh = w_sb[k][:, cs]
                            if USE_F32R:
                                lh, rh = lh.bitcast(F32R), rh.bitcast(F32R)
                            nc.tensor.matmul(pf[:, cs], lh, rh,
                                             start=(k == 0), stop=(k == KT - 1))
                    fs = fullp.tile([128, C2], F32, tag="full", name="full")
                    nc.scalar.activation(fs, pf, AF.Copy)
                    full_sb.append(fs)

                # -------- stage B: all-reduce BN partials ----------------------
                statsr = spool.tile([128, 8], F32, tag="statsr")
                if use_collective:
                    nc.sync.dma_start(cc_in.ap(), stats)
                    nc.gpsimd.collective_compute(
                        "AllReduce", ALU.add,
                        replica_groups=[list(range(n_cores))],
                        ins=[cc_in.ap()], outs=[cc_out.ap()])
                    nc.sync.dma_start(statsr, cc_out.ap())
                else:
                    nc.vector.tensor_copy(statsr, stats)

                # -------- stage C: BN affine factors (tiny) --------------------
                mu = spool.tile([128, 4], F32, tag="mu")
                m2 = spool.tile([128, 4], F32, tag="m2")
                var = spool.tile([128, 4], F32, tag="var")
                rs = spool.tile([128, 4], F32, tag="rs")
                sc_t = spool.tile([128, 4], F32, tag="sc_t")
                sh_t = spool.tile([128, 4], F32, tag="sh_t")
                nc.vector.tensor_scalar(mu, statsr[:, 0:4], 1.0 / NTOT, None, ALU.mult)
                nc.vector.tensor_scalar(m2, statsr[:, 4:8], 1.0 / NTOT, None, ALU.mult)
                nc.vector.tensor_tensor(out=var, in0=mu, in1=mu, op=ALU.mult)
                nc.vector.tensor_tensor(out=var, in0=m2, in1=var, op=ALU.subtract)
                nc.vector.tensor_scalar(var, var, BN_EPS, None, ALU.add)
                nc.vector.reciprocal(rs, var)
                nc.scalar.activation(rs, rs, AF.Sqrt)  # rs = 1/sqrt(var+eps)
                nc.vector.tensor_tensor(out=sc_t, in0=rs, in1=gam_all, op=ALU.mult)
                nc.vector.tensor_tensor(out=sh_t, in0=mu, in1=sc_t, op=ALU.mult)
                nc.vector.tensor_tensor(out=sh_t, in0=bet_all, in1=sh_t, op=ALU.subtract)

                # -------- stage D: exact GELU ----------------------------------
                xaT = {}
                for bi, br in enumerate(("o", "i")):
                    xaT[br] = []
                    for g in range(GT):
                        col = 2 * bi + g
                        xa = ppool.tile([128, tpc], F32, tag=f"xaT{br}{g}",
                                        name=f"xaT{br}{g}")
                        nc.scalar.activation(xa, xcT[br][g], AF.Gelu,
                                             bias=sh_t[:, col:col + 1],
                                             scale=sc_t[:, col:col + 1])
                        xaT[br].append(xa)

                Dcol = {"o": spool.tile([128, NTT], F32, tag="D_o", name="D_o"),
                        "i": spool.tile([128, NTT], F32, tag="D_i", name="D_i")}
                dinv = {"o": spool.tile([128, NTT], F32, tag="dv_o", name="dv_o"),
                        "i": spool.tile([128, NTT], F32, tag="dv_i", name="dv_i")}

                def gating_tile(br, t, ut_tiles):
                    """logits -> unnormalized exp E (SBUF) + D accum, token tile t."""
                    lbr = rows["lb_i"] if br == "i" else rows["lb_o"]
                    pl = ps.tile([128, C2], F32, tag="pacc", name="pl")
                    for ch in range(2):
                        cs = slice(512 * ch, 512 * (ch + 1))
                        for g in range(GT):
                            nc.tensor.matmul(pl[:, cs],
                                             xaT[br][g][:, 128 * t:128 * (t + 1)],
                                             ut_tiles[g][:, cs],
                                             start=(g == 0), stop=False)
                        nc.tensor.matmul(pl[:, cs], ones_row, lbr[:, cs],
                                         start=False, stop=True)
                    e = ep.tile([128, C2], F32, tag=f"E{br}", name=f"E{br}",
                                bufs=(3 if br == "i" else 2))
                    nc.scalar.activation(e, pl, AF.Exp,
                                         accum_out=Dcol[br][:, t:t + 1])
                    nc.vector.reciprocal(dinv[br][:, t:t + 1], Dcol[br][:, t:t + 1])
                    return e

                def topk_mask_scores(e):
                    """3x(max8+match_replace) + final max8; mask = E >= 32nd value.
                    Tie-free on this data (verified: min rank-32/33 gap ~30 ulps)."""
                    scr = wk.tile([128, C2], F32, tag="scr", name="scr")
                    t8 = None
                    for r in range(4):
                        t8 = wk.tile([128, 8], F32, tag="t8", name="t8")
                        src = e if r == 0 else scr
                        nc.vector.max(out=t8, in_=src)
                        if r < 3:
                            nc.vector.match_replace(out=scr, in_to_replace=t8,
                                                    in_values=src, imm_value=0.0)
                    nc.vector.tensor_scalar(scr, e, t8[:, 7:8], None, ALU.is_ge)
                    return scr

                # -------- stage E: in-branch gating, rounds, transposes, xs ----
                E_i = [gating_tile("i", t, ut_i_sb) for t in range(NTT)]

                # D_i as rows [1,128] per t (for folding D*bias into in psum)
                din_row = []
                for t in range(NTT):
                    pdt = ps.tile([128, C2], F32, tag="pacc", name="pdt")[:1, :128]
                    nc.tensor.matmul(pdt, Dcol["i"][:, t:t + 1], ident,
                                     start=True, stop=True)
                    dr = spool.tile([1, 128], F32, tag=f"din{t}", name=f"din{t}")
                    nc.vector.tensor_copy(dr, pdt)
                    din_row.append(dr)

                xs3 = xs_all.rearrange("p (k t) -> p k t", k=KT)
                xT3 = xT_all.rearrange("p (k t) -> p k t", k=KT)
                for t in range(NTT):
                    scr = topk_mask_scores(E_i[t])
                    em = wk.tile([128, C2], F32, tag="em_i", name="em_i")
                    nc.gpsimd.tensor_tensor(out=em, in0=E_i[t], in1=scr,
                                            op=ALU.mult)
                    ptr = pstr.tile([128, C2], F32, tag="ptr", name="ptr")
                    for k in range(KT):
                        nc.tensor.transpose(ptr[:, 128 * k:128 * (k + 1)],
                                            em[:, 128 * k:128 * (k + 1)], ident)
                    p3 = ptr.rearrange("p (k t) -> p k t", k=KT)
                    nc.vector.tensor_tensor(
                        out=xs3[:, :, 128 * t:128 * (t + 1)],
                        in0=xT3[:, :, 128 * t:128 * (t + 1)],
                        in1=p3, op=ALU.mult)

                # load ut_o (reuses ut slots after last logits_i use)
                ut_o_sb = []
                for g in range(GT):
                    t = utpool.tile([128, C2], F32, tag="ut", name=f"ut_o{g}")
                    nc.sync.dma_start(t, ut_o[128 * g:128 * (g + 1), :])
                    ut_o_sb.append(t)

                # -------- stage G: per-token-tile tail -------------------------
                for t in range(NTT):
                    # in-branch matmul + bias fold, scaled by 1/D_i
                    pin = ps.tile([128, C2], F32, tag="pacc", name="pin")
                    for ch in range(2):
                        cs = slice(512 * ch, 512 * (ch + 1))
                        for k in range(KT):
                            lh = xtile(xs_all, k, 128 * t, 128 * (t + 1))
                            rh = w_sb[k][:, cs]
                            if USE_F32R:
                                lh, rh = lh.bitcast(F32R), rh.bitcast(F32R)
                            nc.tensor.matmul(pin[:, cs], lh, rh,
                                             start=(k == 0), stop=False)
                        nc.tensor.matmul(pin[:, cs], din_row[t], rows["bias_r"][:, cs],
                                         start=False, stop=True)
                    f1 = wk.tile([128, C2], F32, tag="f1", name="f1")
                    nc.scalar.activation(f1, pin, AF.Identity,
                                         scale=dinv["i"][:, t:t + 1])

                    # out-branch for this tile
                    e_o = gating_tile("o", t, ut_o_sb)
                    scr = topk_mask_scores(e_o)
                    em = wk.tile([128, C2], F32, tag="em_o", name="em_o")
                    nc.gpsimd.tensor_tensor(out=em, in0=e_o, in1=scr, op=ALU.mult)
                    ems = wk.tile([128, C2], F32, tag="ems", name="ems")
                    nc.gpsimd.tensor_scalar(ems, em, dinv["o"][:, t:t + 1],
                                            None, ALU.mult)
                    osb = wk.tile([128, C2], F32, tag="osb", name="osb")
                    nc.gpsimd.tensor_tensor(out=osb, in0=full_sb[t], in1=ems,
                                            op=ALU.mult)
                    nc.gpsimd.tensor_tensor(out=osb, in0=osb, in1=f1, op=ALU.add)
                    nc.sync.dma_start(out_d[128 * t:128 * (t + 1), :], osb)

    nc.compile()
    return nc


def host_prep(inputs, n_cores=N_CORES, tpc=TPC_PROD):
    """Shard + lay out FULL inputs into per-core in_maps."""
    x = np.ascontiguousarray(np.asarray(inputs["x"], np.float32))
    weight = np.ascontiguousarray(np.asarray(inputs["weight"], np.float32))
    x2d = x.reshape(B * L, C1)

    def scatter_conv(cw):
        # compact per-k-tile layout: ac[p, 32k + p//4] = conv_w[32k + p//4, p%4]
        cw = np.asarray(cw, np.float32)
        ac = np.zeros((128, 256), np.float32)
        p = np.arange(128)
        for k in range(8):
            ac[p, 32 * k + p // 4] = cw[32 * k + p // 4, p % 4]
        return ac

    def pack_gb(a_o, a_i):
        a_o = np.asarray(a_o, np.float32).reshape(2, 128)
        a_i = np.asarray(a_i, np.float32).reshape(2, 128)
        return np.ascontiguousarray(np.stack([a_o[0], a_o[1], a_i[0], a_i[1]]))

    shared = dict(
        w=weight,
        at_o=scatter_conv(inputs["so_conv_w"]),
        at_i=scatter_conv(inputs["si_conv_w"]),
        ut_o=np.ascontiguousarray(np.asarray(inputs["so_lin_w"], np.float32).T),
        ut_i=np.ascontiguousarray(np.asarray(inputs["si_lin_w"], np.float32).T),
        gam=pack_gb(inputs["so_gamma"], inputs["si_gamma"]),
        bet=pack_gb(inputs["so_beta"], inputs["si_beta"]),
        lb_o=np.asarray(inputs["so_lin_b"], np.float32).reshape(1, C2),
        lb_i=np.asarray(inputs["si_lin_b"], np.float32).reshape(1, C1),
        bias_r=np.asarray(inputs["bias"], np.float32).reshape(1, C2),
    )
    # conv_b dropped: BatchNorm is shift-invariant, the conv bias cancels.
    in_maps = []
    for c in range(n_cores):
        m = dict(shared)
        m["xT"] = np.ascontiguousarray(x2d[c * tpc:(c + 1) * tpc].T)
        in_maps.append(m)
    return in_maps


_CACHE = {}


def kernel(**inputs):
    if "prod" not in _CACHE:
        _CACHE["prod"] = build_module()
    nc = _CACHE["prod"]
    in_maps = host_prep(inputs)
    res = run_bass_kernel_spmd(nc, in_maps, core_ids=list(range(N_CORES)))
    full = np.concatenate([r["out"] for r in res.results], axis=0)
    return full.reshape(B, L, C2).astype(np.float32)

